# revision 1
# baseline (speedup 1.0000x reference)
"""Trainium2 Bass kernel for nn_ConformerEncoderLayer (B=16, L=512, D=512, H=8, FF=2048).

Sharding: data-parallel over batch across 8 NeuronCores (2 batch elems / core).
Device layout: feature-major residual stream x_fm [D=512 (4x128 chunks), T=1024].
Matmuls run in float32r (full PE rate at N=512, ~1.5e-4 rel err).
BatchNorm statistics are all-reduced across the 8 cores (ncfw collective,
warmed up by a dummy collective at kernel start).
"""
import sys

sys.path.insert(0, "/opt/trn_rl_repo")

import contextlib
import numpy as np
import ml_dtypes
import concourse.bacc as bacc
import concourse.tile as tile
from concourse import mybir
from concourse.bass_utils import run_bass_kernel_spmd

FP32 = mybir.dt.float32
FP32R = mybir.dt.float32r

N_CORES = 8
B, L, D, H, FF = 16, 512, 512, 8, 2048
HD = D // H                # 64
BL = B // N_CORES          # 2 batch elems per core
T = BL * L                 # 1024 tokens per core
NC = D // 128              # 4 feature chunks
ROPE_BASE = 10000.0
LN_EPS = 1e-5
BN_EPS = 1e-5
KTAP = 31
PAD = 15
CONVW = PAD + L + PAD + L + PAD          # zeros|b0|zeros|b1|zeros = 1069
OFF_B = (PAD, PAD + L + PAD)             # start col of each batch's data

_CACHE = {}


# ---------------------------------------------------------------- host prep

def _prep_host(inp):
    f32 = np.float32
    x = np.asarray(inp["x"], f32)
    ln = {k: np.asarray(inp[k], f32) for k in
          ("ln1_g", "ln1_b", "ln2_g", "ln2_b", "ln3_g", "ln3_b",
           "ln4_g", "ln4_b", "ln5_g", "ln5_b")}

    def colvec(b, n):  # [F] -> [128, F//128]  (tile[p, i] = b[i*128+p])
        return np.ascontiguousarray(np.asarray(b, f32).reshape(n, 128).T)

    w_ff1a = np.asarray(inp["w_ff1a"], f32)
    w_ff1b = np.asarray(inp["w_ff1b"], f32)
    w_ff2a = np.asarray(inp["w_ff2a"], f32)
    w_ff2b = np.asarray(inp["w_ff2b"], f32)

    d = {}
    d["wff1a"] = np.ascontiguousarray((w_ff1a * ln["ln1_g"][None, :]).T)
    d["bff1a"] = colvec(np.asarray(inp["b_ff1a"], f32) + ln["ln1_b"] @ w_ff1a.T, 16)
    d["wff1b"] = np.ascontiguousarray(0.5 * w_ff1b.T)
    d["bff1b"] = colvec(0.5 * np.asarray(inp["b_ff1b"], f32), 4)
    d["wff2a"] = np.ascontiguousarray((w_ff2a * ln["ln4_g"][None, :]).T)
    d["bff2a"] = colvec(np.asarray(inp["b_ff2a"], f32) + ln["ln4_b"] @ w_ff2a.T, 16)
    d["wff2b"] = np.ascontiguousarray(0.5 * w_ff2b.T)
    d["bff2b"] = colvec(0.5 * np.asarray(inp["b_ff2b"], f32), 4)

    w_in = np.asarray(inp["w_in"], f32)
    b_in = np.asarray(inp["b_in"], f32)
    wq, wk, wv = w_in[:D], w_in[D:2 * D], w_in[2 * D:]
    bq, bk, bv = b_in[:D], b_in[D:2 * D], b_in[2 * D:]
    d["wqT"] = np.ascontiguousarray(wq.T)
    d["wkT"] = np.ascontiguousarray(wk.T)
    d["wvT"] = np.ascontiguousarray((wv * ln["ln2_g"][None, :]).T)
    d["bq"] = colvec(bq, 4)
    d["bk"] = colvec(bk, 4)
    d["bv_row"] = np.ascontiguousarray((bv + ln["ln2_b"] @ wv.T).reshape(1, D))
    w_out = np.asarray(inp["w_out"], f32)
    d["woutTh"] = np.ascontiguousarray(
        w_out.T.reshape(H, HD, D).transpose(1, 0, 2))          # [64, 8, 512]
    d["bout"] = colvec(np.asarray(inp["b_out"], f32), 4)

    pos = np.arange(L, dtype=f32)
    num = np.arange(0, D, 2, dtype=f32) / D
    scale = (1.0 / ROPE_BASE ** num).astype(ml_dtypes.bfloat16).astype(f32)
    theta = pos[:, None] * scale[None, :]                      # [512, 256]
    cosT, sinT = np.cos(theta).T, np.sin(theta).T              # [256, 512]
    cos2 = np.concatenate([cosT] * BL, axis=1)                 # [256, 1024]
    sin2 = np.concatenate([sinT] * BL, axis=1)
    g1h = ln["ln2_g"][:D // 2][:, None]
    g2h = ln["ln2_g"][D // 2:][:, None]
    d["ropetab"] = np.ascontiguousarray(np.stack(
        [g1h * cos2, g1h * sin2, g2h * cos2, g2h * sin2]))     # [C1,S1,C2,S2]

    has_qkfix = not np.allclose(ln["ln2_b"], 0.0)
    if has_qkfix:
        bb = np.broadcast_to(ln["ln2_b"], (L, D)).astype(f32)
        half = D // 2
        rb = np.concatenate([bb[:, :half] * cosT.T - bb[:, half:] * sinT.T,
                             bb[:, half:] * cosT.T + bb[:, :half] * sinT.T], axis=1)
        qfix = np.concatenate([(rb @ wq.T).T] * BL, axis=1)
        kfix = np.concatenate([(rb @ wk.T).T] * BL, axis=1)
        d["qkfix"] = np.ascontiguousarray(np.stack([qfix, kfix]))  # [2, 512, 1024]

    w_glu = np.asarray(inp["w_glu"], f32)
    d["wgluT"] = np.ascontiguousarray((w_glu * ln["ln3_g"][None, :]).T)  # [512,1024]
    d["bglu"] = colvec(ln["ln3_b"] @ w_glu.T, 8)
    w_dw = np.asarray(inp["w_dw"], f32)[:, 0, :]               # [512, 31]
    d["wdw"] = np.ascontiguousarray(
        w_dw.reshape(NC, 128, KTAP).transpose(1, 0, 2))        # [128, 4, 31]
    d["bng"] = colvec(np.asarray(inp["bn_g"], f32), 4)
    d["bnb"] = colvec(np.asarray(inp["bn_b"], f32), 4)
    d["wpwT"] = np.ascontiguousarray(np.asarray(inp["w_pw"], f32).T)
    d["eye"] = np.eye(128, dtype=f32)
    d["cconst"] = np.full((128, 1), 1.0 / D, f32)
    d["rowones"] = np.ones((1, 128), f32)
    d["vones"] = np.ones((128, 8, H), f32)
    d["padzero"] = np.zeros((128, NC, PAD), f32)

    ln5_nontrivial = not (np.allclose(ln["ln5_g"], 1.0)
                          and np.allclose(ln["ln5_b"], 0.0))
    if ln5_nontrivial:
        d["g5"] = colvec(ln["ln5_g"], 4)
        d["b5"] = colvec(ln["ln5_b"], 4)

    xs = []
    for c in range(N_CORES):
        xc = x[BL * c: BL * (c + 1)]                           # [2, 512, 512]
        xs.append(np.ascontiguousarray(xc.transpose(2, 0, 1).reshape(D, T)))
    return d, xs, (has_qkfix, ln5_nontrivial)


# ---------------------------------------------------------------- device build

def _build(flags):
    has_qkfix, has_ln5gb = flags
    nc = bacc.Bacc("TRN2", target_bir_lowering=False, debug=False,
                   enable_asserts=True, num_devices=N_CORES)
    AOT = mybir.AluOpType
    AF = mybir.ActivationFunctionType

    def din(name, shape):
        return nc.dram_tensor(name, list(shape), FP32, kind="ExternalInput")

    x_in = din("x_fm", [D, T])
    wff1a_d = din("wff1a", [D, FF]); bff1a_d = din("bff1a", [128, 16])
    wff1b_d = din("wff1b", [FF, D]); bff1b_d = din("bff1b", [128, 4])
    wff2a_d = din("wff2a", [D, FF]); bff2a_d = din("bff2a", [128, 16])
    wff2b_d = din("wff2b", [FF, D]); bff2b_d = din("bff2b", [128, 4])
    wqT_d = din("wqT", [D, D]); wkT_d = din("wkT", [D, D]); wvT_d = din("wvT", [D, D])
    bq_d = din("bq", [128, 4]); bk_d = din("bk", [128, 4]); bv_d = din("bv_row", [1, D])
    woutTh_d = din("woutTh", [HD, H, D]); bout_d = din("bout", [128, 4])
    ropetab_d = din("ropetab", [4, 256, T])
    wgluT_d = din("wgluT", [D, 2 * D]); bglu_d = din("bglu", [128, 8])
    wdw_d = din("wdw", [128, NC, KTAP])
    bng_d = din("bng", [128, 4]); bnb_d = din("bnb", [128, 4])
    wpwT_d = din("wpwT", [D, D])
    eye_d = din("eye", [128, 128])
    cconst_d = din("cconst", [128, 1])
    rowones_d = din("rowones", [1, 128])
    vones_d = din("vones", [128, 8, H])
    padzero_d = din("padzero", [128, NC, PAD])
    qkfix_d = din("qkfix", [2, D, T]) if has_qkfix else None
    g5_d = din("g5", [128, 4]) if has_ln5gb else None
    b5_d = din("b5", [128, 4]) if has_ln5gb else None
    out_d = nc.dram_tensor("out", [BL, L, D], FP32, kind="ExternalOutput")
    out_flat = out_d.ap().rearrange("b l d -> (b l) d")

    def chunked(ap_dram):
        return ap_dram.ap().rearrange("(c p) f -> p c f", p=128)

    with tile.TileContext(nc) as tc:
        ctx = contextlib.ExitStack()
        with ctx:
            resid = ctx.enter_context(tc.tile_pool(name="resid", bufs=1))
            zpool = ctx.enter_context(tc.tile_pool(name="zpool", bufs=1))
            scr = ctx.enter_context(tc.tile_pool(name="scr", bufs=1))
            sqp = ctx.enter_context(tc.tile_pool(name="sqp", bufs=2))
            stat = ctx.enter_context(tc.tile_pool(name="stat", bufs=1))
            bias1 = ctx.enter_context(tc.tile_pool(name="bias1", bufs=1))
            dpool = ctx.enter_context(tc.tile_pool(name="dpool", bufs=4, space="DRAM"))

            # ---------------- persistent tiles ----------------
            x = resid.tile([128, NC, T], FP32R, tag="x")
            for _c in range(NC):
                nc.sync.dma_start(x[:, _c, :], chunked(x_in).bitcast(FP32R)[:, _c, :])
            oneD_r = bias1.tile([128, 1], FP32R, tag="oneD_r")
            nc.sync.dma_start(oneD_r[:], cconst_d.ap().bitcast(FP32R))
            ones_row_r = bias1.tile([1, 128], FP32R, tag="ones_row")
            nc.sync.dma_start(ones_row_r[:], rowones_d.ap().bitcast(FP32R))
            eye_sb = bias1.tile([128, 128], FP32, tag="eye")
            nc.sync.dma_start(eye_sb[:], eye_d.ap())
            eye_r = bias1.tile([128, 128], FP32R, tag="eye_r")
            nc.sync.dma_start(eye_r[:], eye_d.ap().bitcast(FP32R))
            eps_sb = bias1.tile([128, 1], FP32, tag="eps")
            nc.vector.memset(eps_sb[:], LN_EPS)

            # ---------------- collective warm-up ----------------
            warm_sb = bias1.tile([128, 8], FP32, tag="warm")
            nc.vector.memset(warm_sb[:], 0.0)
            warm_in = dpool.tile([128, 8], FP32)
            warm_out = dpool.tile([128, 8], FP32)
            nc.gpsimd.dma_start(warm_in[:], warm_sb[:])
            nc.gpsimd.collective_compute(
                "AllReduce", AOT.add, replica_groups=[list(range(N_CORES))],
                ins=[warm_in.opt()], outs=[warm_out.opt()])
            nc.gpsimd.dma_start(warm_sb[:], warm_out[:])

            # ---------------- helpers ----------------
            def layer_norm(tag):
                """z = (x - mean)/std over features, feature-major, fp32r."""
                z = zpool.tile([128, NC, T], FP32R, tag="z")
                with tc.tile_pool(name="psln" + tag, bufs=1, space="PSUM") as psln:
                    mean_ps = psln.tile([1, 2, 512], FP32, tag="mean")
                    msq_ps = psln.tile([1, 2, 512], FP32, tag="msq")
                    for nh in range(2):
                        for c in range(NC):
                            sq = sqp.tile([128, 512], FP32R, tag="sq")
                            nc.scalar.activation(
                                out=sq[:], in_=x[:, c, 512 * nh:512 * (nh + 1)].bitcast(FP32),
                                func=AF.Square)
                            nc.tensor.matmul(mean_ps[:, nh, :], oneD_r[:],
                                             x[:, c, 512 * nh:512 * (nh + 1)],
                                             start=(c == 0), stop=(c == NC - 1))
                            nc.tensor.matmul(msq_ps[:, nh, :], oneD_r[:], sq[:],
                                             start=(c == 0), stop=(c == NC - 1))
                    s_r = stat.tile([1, 2, 512], FP32, tag="s_r")
                    s_mr = stat.tile([1, 2, 512], FP32, tag="s_mr")
                    m2 = stat.tile([1, 2, 512], FP32, tag="m2")
                    nc.scalar.activation(out=m2[:], in_=mean_ps[:], func=AF.Square)
                    nc.vector.tensor_tensor(out=m2[:], in0=msq_ps[:], in1=m2[:],
                                            op=AOT.subtract)
                    nc.scalar.activation(out=m2[:], in_=m2[:], func=AF.Sqrt,
                                         bias=eps_sb[0:1, :], scale=1.0)
                    nc.vector.reciprocal(out=s_r[:], in_=m2[:])
                    nc.vector.tensor_tensor(out=s_mr[:], in0=mean_ps[:], in1=s_r[:],
                                            op=AOT.mult)
                    r_b = scr.tile([128, 2, 512], FP32, tag="r_b")
                    mr_b = scr.tile([128, 2, 512], FP32, tag="mr_b")
                    r_bd = dpool.tile([1, 2, 512], FP32, tag="r_bd")
                    mr_bd = dpool.tile([1, 2, 512], FP32, tag="mr_bd")
                    nc.sync.dma_start(r_bd[:], s_r[:])
                    nc.sync.dma_start(mr_bd[:], s_mr[:])
                    nc.sync.dma_start(r_b[:], r_bd[:].to_broadcast((128, 2, 512)))
                    nc.sync.dma_start(mr_b[:], mr_bd[:].to_broadcast((128, 2, 512)))
                    tmp = scr.tile([128, T], FP32, tag="lntmp")
                    for c in range(NC):
                        nc.vector.tensor_tensor(
                            out=tmp[:], in0=x[:, c, :].bitcast(FP32),
                            in1=r_b[:].rearrange("p a b -> p (a b)"), op=AOT.mult)
                        nc.vector.tensor_tensor(
                            out=z[:, c, :], in0=tmp[:],
                            in1=mr_b[:].rearrange("p a b -> p (a b)"), op=AOT.subtract)
                return z

            def ffn(z, wa_d, ba_d, wb_d, bb_d, tag):
                with tc.tile_pool(name="wp" + tag, bufs=1) as wpool, \
                     tc.tile_pool(name="hp" + tag, bufs=1) as hpool, \
                     tc.tile_pool(name="psf" + tag, bufs=3, space="PSUM") as psum:
                    wa = wpool.tile([128, NC, FF], FP32R, tag="wa")
                    for _m in range(0, 16, 2):
                        nc.sync.dma_start(
                            wa[:, :, 128 * _m:128 * (_m + 2)],
                            chunked(wa_d).bitcast(FP32R)[:, :, 128 * _m:128 * (_m + 2)])
                    wb = wpool.tile([128, 16, D], FP32R, tag="wb")
                    _wbap = wb_d.ap().rearrange("(c p) f -> p c f", p=128).bitcast(FP32R)
                    for _k in range(0, 16, 2):
                        nc.sync.dma_start(wb[:, _k:_k + 2, :], _wbap[:, _k:_k + 2, :])
                    ba = bias1.tile([128, 16], FP32, tag="ba" + tag)
                    nc.sync.dma_start(ba[:], ba_d.ap())
                    bb = bias1.tile([128, 4], FP32, tag="bb" + tag)
                    nc.sync.dma_start(bb[:], bb_d.ap())
                    for nh in range(2):
                        hid = hpool.tile([128, 16, 512], FP32R, tag="hid")
                        for mt in range(16):
                            p = psum.tile([128, 512], FP32, tag="mm")
                            for kc in range(NC):
                                nc.tensor.matmul(
                                    p[:], wa[:, kc, 128 * mt:128 * (mt + 1)],
                                    z[:, kc, 512 * nh:512 * (nh + 1)],
                                    start=(kc == 0), stop=(kc == NC - 1))
                            nc.scalar.activation(
                                out=hid[:, mt, :], in_=p[:],
                                func=AF.Relu, bias=ba[:, mt:mt + 1], scale=1.0)
                        for mc in range(NC):
                            p = psum.tile([128, 512], FP32, tag="mm")
                            for kt in range(16):
                                nc.tensor.matmul(
                                    p[:], wb[:, kt, 128 * mc:128 * (mc + 1)],
                                    hid[:, kt, :],
                                    start=(kt == 0), stop=(kt == 15))
                            nc.vector.scalar_tensor_tensor(
                                out=x[:, mc, 512 * nh:512 * (nh + 1)],
                                in0=p[:], scalar=bb[:, mc:mc + 1],
                                in1=x[:, mc, 512 * nh:512 * (nh + 1)].bitcast(FP32),
                                op0=AOT.add, op1=AOT.add)

            # ================= FFN1 =================
            with nc.named_scope("ffn1"):
                z1 = layer_norm("1")
                ffn(z1, wff1a_d, bff1a_d, wff1b_d, bff1b_d, "1")

            # ================= attention =================
            with nc.named_scope("attn"):
                z2 = layer_norm("2")
                with tc.tile_pool(name="apool", bufs=1) as apool, \
                     tc.tile_pool(name="psa", bufs=4, space="PSUM") as psum, \
                     tc.tile_pool(name="psav", bufs=4, space="PSUM") as psav:
                    # ---- v projection (token-major, ones-augmented) ----
                    v_aug = apool.tile([128, 8, H, HD + 1], FP32R, tag="vaug")
                    nc.sync.dma_start(v_aug[:, :, :, HD:HD + 1],
                                      vones_d.ap().bitcast(FP32R)[:, :, :, None])
                    with tc.tile_pool(name="wvp", bufs=1) as wvp:
                        wv_sb = wvp.tile([128, NC, D], FP32R, tag="wv")
                        nc.sync.dma_start(wv_sb[:], chunked(wvT_d).bitcast(FP32R))
                        bvr = bias1.tile([1, D], FP32R, tag="bvr")
                        nc.sync.dma_start(bvr[:], bv_d.ap().bitcast(FP32R))
                        for tt in range(8):
                            p = psum.tile([128, 512], FP32, tag="mm")
                            for kc in range(NC):
                                nc.tensor.matmul(p[:], z2[:, kc, 128 * tt:128 * (tt + 1)],
                                                 wv_sb[:, kc, :],
                                                 start=(kc == 0), stop=False)
                            nc.tensor.matmul(p[:], ones_row_r[:], bvr[:],
                                             start=False, stop=True)
                            nc.scalar.copy(out=v_aug[:, tt, :, 0:HD],
                                           in_=p[:].rearrange("p (h f) -> p h f", h=H))
                    # ---- rope + q/k ----
                    q_sb = apool.tile([128, NC, T], FP32R, tag="q")
                    k_sb = apool.tile([128, NC, T], FP32R, tag="k")
                    if has_qkfix:
                        qkf = apool.tile([128, 2, NC, T], FP32, tag="qkf")
                        nc.sync.dma_start(
                            qkf[:], qkfix_d.ap().rearrange("k (c p) t -> p k c t", p=128))
                    with tc.tile_pool(name="hrp", bufs=1) as hrp:
                        hr = hrp.tile([128, NC, T], FP32R, tag="hr")
                        with tc.tile_pool(name="tabp", bufs=1) as tabp:
                            tab = tabp.tile([128, 4, 2, T], FP32, tag="ropetab")
                            nc.sync.dma_start(
                                tab[:],
                                ropetab_d.ap().rearrange("k (c p) t -> p k c t", p=128))
                            rtmp = tabp.tile([128, T], FP32, tag="rtmp")
                            rtmp2 = tabp.tile([128, T], FP32, tag="rtmp2")
                            for c in range(2):
                                nc.vector.tensor_tensor(
                                    out=rtmp[:], in0=z2[:, c, :].bitcast(FP32),
                                    in1=tab[:, 0, c, :], op=AOT.mult)
                                nc.vector.tensor_tensor(
                                    out=rtmp2[:], in0=z2[:, c + 2, :].bitcast(FP32),
                                    in1=tab[:, 3, c, :], op=AOT.mult)
                                nc.vector.tensor_tensor(
                                    out=hr[:, c, :], in0=rtmp[:], in1=rtmp2[:],
                                    op=AOT.subtract)
                                nc.vector.tensor_tensor(
                                    out=rtmp[:], in0=z2[:, c + 2, :].bitcast(FP32),
                                    in1=tab[:, 2, c, :], op=AOT.mult)
                                nc.vector.tensor_tensor(
                                    out=rtmp2[:], in0=z2[:, c, :].bitcast(FP32),
                                    in1=tab[:, 1, c, :], op=AOT.mult)
                                nc.vector.tensor_tensor(
                                    out=hr[:, c + 2, :], in0=rtmp[:], in1=rtmp2[:],
                                    op=AOT.add)
                        with tc.tile_pool(name="wqkp", bufs=1) as wqkp:
                            wq_sb = wqkp.tile([128, NC, D], FP32R, tag="wq")
                            wk_sb = wqkp.tile([128, NC, D], FP32R, tag="wk")
                            nc.sync.dma_start(wq_sb[:], chunked(wqT_d).bitcast(FP32R))
                            nc.sync.dma_start(wk_sb[:], chunked(wkT_d).bitcast(FP32R))
                            bqs = bias1.tile([128, 4], FP32, tag="bqs")
                            bks = bias1.tile([128, 4], FP32, tag="bks")
                            nc.sync.dma_start(bqs[:], bq_d.ap())
                            nc.sync.dma_start(bks[:], bk_d.ap())
                            for dst, w_sb, bias_sb, fi in ((q_sb, wq_sb, bqs, 0),
                                                           (k_sb, wk_sb, bks, 1)):
                                for mt in range(NC):
                                    for nh in range(2):
                                        p = psum.tile([128, 512], FP32, tag="mm")
                                        for kc in range(NC):
                                            nc.tensor.matmul(
                                                p[:], w_sb[:, kc, 128 * mt:128 * (mt + 1)],
                                                hr[:, kc, 512 * nh:512 * (nh + 1)],
                                                start=(kc == 0), stop=(kc == NC - 1))
                                        if has_qkfix:
                                            nc.vector.scalar_tensor_tensor(
                                                out=dst[:, mt, 512 * nh:512 * (nh + 1)],
                                                in0=p[:], scalar=bias_sb[:, mt:mt + 1],
                                                in1=qkf[:, fi, mt,
                                                        512 * nh:512 * (nh + 1)],
                                                op0=AOT.add, op1=AOT.add)
                                        else:
                                            nc.scalar.activation(
                                                out=dst[:, mt, 512 * nh:512 * (nh + 1)],
                                                in_=p[:], func=AF.Identity,
                                                bias=bias_sb[:, mt:mt + 1], scale=1.0)
                    # ---- scores / softmax / AV / out-proj ----
                    with tc.tile_pool(name="ep", bufs=3) as epool, \
                         tc.tile_pool(name="wop", bufs=1) as wop, \
                         tc.tile_pool(name="osbp", bufs=1) as osbp:
                        wo_sb = wop.tile([HD, H, D], FP32R, tag="wo")
                        nc.sync.dma_start(wo_sb[:], woutTh_d.ap().bitcast(FP32R))
                        bo_sb = bias1.tile([128, 4], FP32, tag="bo")
                        nc.sync.dma_start(bo_sb[:], bout_d.ap())
                        for b in range(BL):
                            o_sb = osbp.tile([HD, H, 512], FP32R, tag="osb")
                            for h in range(H):
                                pb, ch = HD * (h % 2), h // 2
                                e_t = epool.tile([128, 4, 512], FP32R, tag="e")
                                for kt in range(4):
                                    s_ps = psum.tile([128, 512], FP32, tag="mm")
                                    nc.tensor.matmul(
                                        s_ps[:],
                                        k_sb[pb:pb + HD, ch,
                                             512 * b + 128 * kt:512 * b + 128 * (kt + 1)],
                                        q_sb[pb:pb + HD, ch, 512 * b:512 * (b + 1)],
                                        start=True, stop=True)
                                    nc.scalar.activation(out=e_t[:, kt, :], in_=s_ps[:],
                                                         func=AF.Exp, scale=1.0 / 8.0)
                                o_ps = psav.tile([HD + 1, 512], FP32, tag="avo")
                                for kt in range(4):
                                    nc.tensor.matmul(o_ps[:], v_aug[:, 4 * b + kt, h, :],
                                                     e_t[:, kt, :],
                                                     start=(kt == 0), stop=(kt == 3))
                                invd = sqp.tile([HD + 1, 512], FP32, tag="invd")
                                nc.vector.reciprocal(out=invd[HD:HD + 1, :],
                                                     in_=o_ps[HD:HD + 1, :])
                                invb = sqp.tile([HD, 512], FP32, tag="invb")
                                invd_d = dpool.tile([1, 512], FP32, tag="invd_d")
                                nc.sync.dma_start(invd_d[:], invd[HD:HD + 1, :])
                                nc.sync.dma_start(
                                    invb[:], invd_d[:].to_broadcast((HD, 512)))
                                nc.vector.tensor_tensor(out=o_sb[:, h, :],
                                                        in0=o_ps[0:HD, :], in1=invb[:],
                                                        op=AOT.mult)
                            for mc in range(NC):
                                p = psum.tile([128, 512], FP32, tag="mm")
                                for h in range(H):
                                    nc.tensor.matmul(
                                        p[:], wo_sb[:, h, 128 * mc:128 * (mc + 1)],
                                        o_sb[:, h, :],
                                        start=(h == 0), stop=(h == H - 1))
                                nc.vector.scalar_tensor_tensor(
                                    out=x[:, mc, 512 * b:512 * (b + 1)],
                                    in0=p[:], scalar=bo_sb[:, mc:mc + 1],
                                    in1=x[:, mc, 512 * b:512 * (b + 1)].bitcast(FP32),
                                    op0=AOT.add, op1=AOT.add)

            # ================= conv module =================
            with nc.named_scope("conv"):
                z3 = layer_norm("3")
                with tc.tile_pool(name="cpool", bufs=1) as cpool:
                    conv_in = cpool.tile([128, NC, CONVW], FP32R, tag="cin")
                    pz = padzero_d.ap().bitcast(FP32R)
                    nc.sync.dma_start(conv_in[:, :, 0:PAD], pz)
                    nc.sync.dma_start(conv_in[:, :, PAD + L:2 * PAD + L], pz)
                    nc.sync.dma_start(conv_in[:, :, 2 * PAD + 2 * L:CONVW], pz)
                    with tc.tile_pool(name="glup", bufs=1) as glup, \
                         tc.tile_pool(name="wpg", bufs=1) as wpool, \
                         tc.tile_pool(name="psg", bufs=4, space="PSUM") as psum:
                        glu_a = glup.tile([128, NC, T], FP32, tag="glua")
                        glu_s = glup.tile([128, NC, T], FP32, tag="glus")
                        wg_sb = wpool.tile([128, NC, 2 * D], FP32R, tag="wg")
                        nc.sync.dma_start(wg_sb[:], chunked(wgluT_d).bitcast(FP32R))
                        bg_sb = bias1.tile([128, 8], FP32, tag="bg")
                        nc.sync.dma_start(bg_sb[:], bglu_d.ap())
                        for mt in range(8):
                            for nh in range(2):
                                p = psum.tile([128, 512], FP32, tag="mm")
                                for kc in range(NC):
                                    nc.tensor.matmul(
                                        p[:], wg_sb[:, kc, 128 * mt:128 * (mt + 1)],
                                        z3[:, kc, 512 * nh:512 * (nh + 1)],
                                        start=(kc == 0), stop=(kc == NC - 1))
                                if mt < 4:
                                    nc.scalar.activation(
                                        out=glu_a[:, mt, 512 * nh:512 * (nh + 1)],
                                        in_=p[:], func=AF.Identity,
                                        bias=bg_sb[:, mt:mt + 1], scale=1.0)
                                else:
                                    nc.scalar.activation(
                                        out=glu_s[:, mt - 4, 512 * nh:512 * (nh + 1)],
                                        in_=p[:], func=AF.Sigmoid,
                                        bias=bg_sb[:, mt:mt + 1], scale=1.0)
                        for c in range(NC):
                            for b in range(BL):
                                nc.vector.tensor_tensor(
                                    out=conv_in[:, c, OFF_B[b]:OFF_B[b] + L],
                                    in0=glu_a[:, c, L * b:L * (b + 1)],
                                    in1=glu_s[:, c, L * b:L * (b + 1)], op=AOT.mult)
                    wdw_sb = bias1.tile([128, NC, KTAP], FP32, tag="wdw")
                    nc.sync.dma_start(wdw_sb[:], wdw_d.ap())
                    silu_in = cpool.tile([128, NC, T], FP32, tag="siluin")
                    sil = cpool.tile([128, NC, T], FP32R, tag="silu")
                    bng_sb = bias1.tile([128, 4], FP32, tag="bngw")
                    bnb_sb = bias1.tile([128, 4], FP32, tag="bnbw")
                    nc.sync.dma_start(bng_sb[:], bng_d.ap())
                    nc.sync.dma_start(bnb_sb[:], bnb_d.ap())
                    with tc.tile_pool(name="psc", bufs=8, space="PSUM") as psum, \
                         tc.tile_pool(name="diagp", bufs=2) as diagp:
                        stats_loc = stat.tile([128, NC, 2], FP32, tag="bnloc")
                        conv_ps = {}
                        for c in range(NC):
                            diag = diagp.tile([128, KTAP, 128], FP32R, tag="diag")
                            for j in range(KTAP):
                                nc.vector.tensor_scalar_mul(
                                    out=diag[:, j, :], in0=eye_sb[:],
                                    scalar1=wdw_sb[:, c, j:j + 1])
                            for b in range(BL):
                                cp = psum.tile([128, 512], FP32, tag="cps")
                                conv_ps[(c, b)] = cp
                                for j in range(KTAP):
                                    nc.tensor.matmul(
                                        cp[:], diag[:, j, :],
                                        conv_in[:, c,
                                                OFF_B[b] - PAD + j:OFF_B[b] - PAD + j + L],
                                        start=(j == 0), stop=(j == KTAP - 1))
                        st6 = stat.tile([128, NC, 2, 6], FP32, tag="st6")
                        for c in range(NC):
                            for b in range(BL):
                                nc.vector.bn_stats(out=st6[:, c, b, :],
                                                   in_=conv_ps[(c, b)][:])
                            mv = stat.tile([128, 2], FP32, tag="mv")
                            nc.vector.bn_aggr(out=mv[:], in_=st6[:, c, :, :])
                            nc.vector.tensor_scalar_mul(out=stats_loc[:, c, 0:1],
                                                        in0=mv[:, 0:1], scalar1=float(T))
                            m2c = stat.tile([128, 1], FP32, tag="m2c")
                            nc.vector.tensor_tensor(out=m2c[:], in0=mv[:, 0:1],
                                                    in1=mv[:, 0:1], op=AOT.mult)
                            nc.vector.tensor_tensor(out=m2c[:], in0=mv[:, 1:2],
                                                    in1=m2c[:], op=AOT.add)
                            nc.vector.tensor_scalar_mul(out=stats_loc[:, c, 1:2],
                                                        in0=m2c[:], scalar1=float(T))
                        stats_adj = stat.tile([128, 8], FP32, tag="bnadj")
                        nc.vector.tensor_tensor(
                            out=stats_adj[:],
                            in0=stats_loc[:].rearrange("p a b -> p (a b)"),
                            in1=warm_sb[:], op=AOT.add)
                        cc_in = dpool.tile([128, 8], FP32)
                        cc_out = dpool.tile([128, 8], FP32)
                        nc.gpsimd.dma_start(cc_in[:], stats_adj[:])
                        nc.gpsimd.collective_compute(
                            "AllReduce", AOT.add, replica_groups=[list(range(N_CORES))],
                            ins=[cc_in.opt()], outs=[cc_out.opt()])
                        gstats = stat.tile([128, NC, 2], FP32, tag="bngl")
                        nc.gpsimd.dma_start(gstats[:].rearrange("p a b -> p (a b)"),
                                            cc_out[:])
                        ntot = float(N_CORES * T)
                        for c in range(NC):
                            gm = stat.tile([128, 1], FP32, tag="gm")
                            nc.vector.tensor_scalar_mul(out=gm[:], in0=gstats[:, c, 0:1],
                                                        scalar1=1.0 / ntot)
                            gm2 = stat.tile([128, 1], FP32, tag="gm2")
                            nc.vector.tensor_tensor(out=gm2[:], in0=gm[:], in1=gm[:],
                                                    op=AOT.mult)
                            gvar = stat.tile([128, 1], FP32, tag="gvar")
                            nc.vector.scalar_tensor_tensor(
                                out=gvar[:], in0=gstats[:, c, 1:2], scalar=1.0 / ntot,
                                in1=gm2[:], op0=AOT.mult, op1=AOT.subtract)
                            nc.scalar.activation(out=gvar[:], in_=gvar[:], func=AF.Sqrt,
                                                 bias=eps_sb[:], scale=1.0)
                            sfac = stat.tile([128, 1], FP32, tag="sfac")
                            nc.vector.reciprocal(out=sfac[:], in_=gvar[:])
                            nc.vector.tensor_tensor(out=sfac[:], in0=sfac[:],
                                                    in1=bng_sb[:, c:c + 1], op=AOT.mult)
                            for b in range(BL):
                                nc.vector.tensor_scalar(
                                    out=silu_in[:, c, L * b:L * (b + 1)],
                                    in0=conv_ps[(c, b)][:],
                                    scalar1=gm[:], scalar2=sfac[:],
                                    op0=AOT.subtract, op1=AOT.mult)
                            nc.scalar.activation(out=sil[:, c, :], in_=silu_in[:, c, :],
                                                 func=AF.Silu, bias=bnb_sb[:, c:c + 1],
                                                 scale=1.0)
                    with tc.tile_pool(name="wpp", bufs=1) as wpool, \
                         tc.tile_pool(name="psp", bufs=4, space="PSUM") as psum:
                        wpw_sb = wpool.tile([128, NC, D], FP32R, tag="wpw")
                        nc.sync.dma_start(wpw_sb[:], chunked(wpwT_d).bitcast(FP32R))
                        for mc in range(NC):
                            for nh in range(2):
                                p = psum.tile([128, 512], FP32, tag="mm")
                                for kc in range(NC):
                                    nc.tensor.matmul(
                                        p[:], wpw_sb[:, kc, 128 * mc:128 * (mc + 1)],
                                        sil[:, kc, 512 * nh:512 * (nh + 1)],
                                        start=(kc == 0), stop=(kc == NC - 1))
                                nc.vector.scalar_tensor_tensor(
                                    out=x[:, mc, 512 * nh:512 * (nh + 1)],
                                    in0=p[:], scalar=0.0,
                                    in1=x[:, mc, 512 * nh:512 * (nh + 1)].bitcast(FP32),
                                    op0=AOT.add, op1=AOT.add)

            # ================= FFN2 =================
            with nc.named_scope("ffn2"):
                z4 = layer_norm("4")
                ffn(z4, wff2a_d, bff2a_d, wff2b_d, bff2b_d, "2")

            # ================= LN5 + transpose out =================
            with nc.named_scope("ln5out"):
                z5 = layer_norm("5")
                if has_ln5gb:
                    g5s = bias1.tile([128, 4], FP32, tag="g5")
                    b5s = bias1.tile([128, 4], FP32, tag="b5")
                    nc.sync.dma_start(g5s[:], g5_d.ap())
                    nc.sync.dma_start(b5s[:], b5_d.ap())
                    for c in range(NC):
                        nc.vector.tensor_scalar(
                            out=z5[:, c, :], in0=z5[:, c, :].bitcast(FP32),
                            scalar1=g5s[:, c:c + 1], scalar2=b5s[:, c:c + 1],
                            op0=AOT.mult, op1=AOT.add)
                with tc.tile_pool(name="pst", bufs=4, space="PSUM") as psum, \
                     tc.tile_pool(name="outp", bufs=1) as outp:
                    out_sb = outp.tile([128, 8, NC, 128], FP32, tag="outsb")
                    for tt in range(8):
                        for c in range(NC):
                            tp = psum.tile([128, 128], FP32R, tag="tp")
                            nc.tensor.transpose(
                                tp[:], z5[:, c, 128 * tt:128 * (tt + 1)], eye_r[:])
                            nc.scalar.copy(out=out_sb[:, tt, c, :],
                                           in_=tp[:].bitcast(FP32))
                        nc.sync.dma_start(
                            out_flat[128 * tt:128 * (tt + 1), :],
                            out_sb[:, tt, :, :].rearrange("p c f -> p (c f)"))

    nc.compile()
    return nc


# ---------------------------------------------------------------- entry point

def kernel(**inputs):
    d, xs, flags = _prep_host(inputs)
    if flags not in _CACHE:
        _CACHE[flags] = _build(flags)
    nc = _CACHE[flags]
    in_maps = [dict(d, x_fm=xs[c]) for c in range(N_CORES)]
    res = run_bass_kernel_spmd(nc, in_maps, core_ids=list(range(N_CORES)))
    out = np.concatenate([res.results[c]["out"] for c in range(N_CORES)], axis=0)
    return np.ascontiguousarray(out.astype(np.float32))


def run_traced(**inputs):
    """test-only helper: returns (out, BassKernelResults-with-trace)."""
    import ntff_shim
    ntff_shim.install()
    d, xs, flags = _prep_host(inputs)
    if flags not in _CACHE:
        _CACHE[flags] = _build(flags)
    nc = _CACHE[flags]
    in_maps = [dict(d, x_fm=xs[c]) for c in range(N_CORES)]
    res = run_bass_kernel_spmd(nc, in_maps, core_ids=list(range(N_CORES)), trace=True)
    out = np.concatenate([res.results[c]["out"] for c in range(N_CORES)], axis=0)
    return np.ascontiguousarray(out.astype(np.float32)), res

